# revision 8
# baseline (speedup 1.0000x reference)
"""AttentionBlock Trainium2 kernel.

Reference computation (B=16, C=512, H=W=32, n_heads=4, d_k=128):
    xs   = x.reshape(B,C,S).T            # [B, S, C],  S = 1024
    qkv  = xs @ w_proj.T + b_proj        # [B, S, 1536]
    S_   = einsum('bihd,bjhd->bijh', q, k) * d_k**-0.5
    attn = softmax(S_, axis=1)           # over the QUERY axis i (source quirk)
    res  = einsum('bijh,bjhd->bihd', attn, v)
    out  = res @ w_out.T + b_out + xs    # residual
    return out.T.reshape(B, C, H, W)

Data-parallel over batch, 2 batches per core on 8 cores.

v2 design (vs the f32r baseline): the scores (QK^T) and AV matmuls run in
fp8(e4m3) DoubleRow mode (256-wide contraction, 0.5 PE cycles/row), while
the QKV / output projections stay float32r. The softmax exp runs on the ACT
engine only (64 x [128,1024] Exp instructions per core, ~78us) and is the
critical resource; the kernel is emitted as a software pipeline of 64
"steps" (one exp each) with the projection matmuls statically scheduled
into the PE gaps so ACT never starves.

Numerics: exp is computed as exp(s*scale - 3) (softmax shift-invariance
keeps fp8 |e| <= ~40); v rows are pre-scaled by 1024 on the host so
v*inv_den stays in fp8 range, and w_out is pre-divided by 1024. Measured
rel err (numpy sim of the same quantization): ~9e-3 vs the 2e-2 gate.

Layouts (all transposed so no on-device transposes are needed):
  QK^T proj:  psum[f_tile, s] = w_qkT[c, f_tile].T @ x[c, s]   -> qk_sb fp8
  reshuffle:  SB->SB DMA splits d into two 64-partition planes (qk8) for
              the 128-contraction fp8 DoubleRow scores
  V proj:     psum[s_tile, f] = x[c, s_tile].T @ w_vT[c, f]    -> v_sb bf16
  scores:     psum[j, i] = k8[d/2, 2, j].T @ q8[d/2, 2, i]     (DoubleRow)
  exp:        ACT Exp scale=d_k**-0.5 bias=-3, accum_out -> den; e8 fp8
  AV:         racc[d, i] += v8[j, 2, d].T @ e8[j, 2, i]        (DoubleRow)
  out proj:   psum[c_tile, s] = w_outT[f, c_tile].T @ resT[f, s]
              out = (psum + b_out) + x  (one scalar_tensor_tensor)
"""
import sys

for _p in (
    "/opt/trn_rl_repo",
    "/root/.axon_site",
    "/root/.axon_site/_ro/trn_rl_repo",
    "/root/.axon_site/_ro/pypackages",
):
    if _p not in sys.path:
        sys.path.append(_p)

import numpy as np

B = 16
C = 512
S = 1024  # H*W
NH = 4
DK = 128
F = NH * DK  # 512
NCORES = 8
BL = B // NCORES  # batches per core
KT = C // 128  # 4  contraction tiles over channels
ST = S // 128  # 8  seq tiles
NT = S // 512  # 2  free-dim chunks of 512
SCALE = float(DK) ** -0.5
ESHIFT = -3.0  # exp(s*SCALE + ESHIFT): keeps fp8 e <= ~40 (cancels in softmax)
VSC = 1024.0  # host scales w_v by VSC, w_out by 1/VSC; keeps v*inv_den in fp8 range

_CACHE: dict = {}


def _build(repeat=1):
    """Build the kernel. repeat>1 wraps the per-call workload in an on-device
    For_i loop (timing only: amortizes the ~10ms axon dispatch)."""
    import contextlib

    import concourse.bass as bass
    import concourse.tile as tile
    from concourse import bacc, mybir

    F32 = mybir.dt.float32
    F32R = mybir.dt.float32r
    BF16 = mybir.dt.bfloat16
    FP8 = mybir.dt.float8e4
    EXP = mybir.ActivationFunctionType.Exp
    DR = mybir.MatmulPerfMode.DoubleRow
    ADD = mybir.AluOpType.add
    ts = bass.ts

    nc = bacc.Bacc("TRN2", debug=False)
    x_d = nc.dram_tensor("x", [BL, C, S], F32, kind="ExternalInput").ap()
    wqk_d = nc.dram_tensor("w_qkT", [C, 2 * F], F32, kind="ExternalInput").ap()
    wv_d = nc.dram_tensor("w_vT", [C, F], F32, kind="ExternalInput").ap()
    wo_d = nc.dram_tensor("w_outT", [F, C], F32, kind="ExternalInput").ap()
    bias_d = nc.dram_tensor("bias", [128, 2 * NH + F + KT + 1], F32, kind="ExternalInput").ap()
    out_d = nc.dram_tensor("out", [BL, C, S], F32, kind="ExternalOutput").ap()

    xr = x_d.bitcast(F32R)
    wqk_r = wqk_d.rearrange("(k p) m -> p k m", p=128).bitcast(F32R)
    wv_r = wv_d.rearrange("(k p) m -> p k m", p=128).bitcast(F32R)
    wo_r = wo_d.rearrange("(k p) m -> p k m", p=128).bitcast(F32R)

    with tile.TileContext(nc) as tc:
        with (
            tc.tile_pool(name="const", bufs=1) as constp,
            tc.tile_pool(name="xp", bufs=2) as xp,
            tc.tile_pool(name="qkp", bufs=2) as qkp,
            tc.tile_pool(name="qrp", bufs=2) as qrp,
            tc.tile_pool(name="vp", bufs=2) as vp,
            tc.tile_pool(name="rp", bufs=2) as rp,
            tc.tile_pool(name="ep", bufs=3) as epool,
            tc.tile_pool(name="v8p", bufs=4) as v8pool,
            tc.tile_pool(name="small", bufs=8) as smallp,
            tc.tile_pool(name="otp", bufs=4) as otp,
            # psum: pp = [128,512]x2 proj accumulators; ps = [128,1024]x2
            # score tiles; pr = [128,1024]x1 AV accumulator. 2+4+2 = 8 banks.
            tc.tile_pool(name="pp", bufs=2, space="PSUM") as pp,
            tc.tile_pool(name="ps", bufs=2, space="PSUM") as ps,
            tc.tile_pool(name="pr", bufs=1, space="PSUM") as pr,
        ):
            # ---- constants + inputs ----
            wqk_sb = constp.tile([128, KT, 2 * F], F32R)
            wv_sb = constp.tile([128, KT, F], F32R)
            wo_sb = constp.tile([128, KT, C], F32R)
            bias_sb = constp.tile([128, 2 * NH + F + KT + 1], F32)
            x_sbs = [xp.tile([128, KT, S], F32R, name=f"x{b}", tag="x") for b in range(BL)]

            # DMA order: bias first (gates prologue DVE), then x[0]/wqk for
            # the first head's projection, wv, the rest, wo last.
            nc.sync.dma_start(out=bias_sb, in_=bias_d)
            for k in range(KT):
                nc.sync.dma_start(out=x_sbs[0][:, k, 0:512], in_=xr[0, bass_ts(k, 128), 0:512])
                nc.sync.dma_start(out=wqk_sb[:, k, 0:256], in_=wqk_r[:, k, 0:256])
            for k in range(KT):
                nc.sync.dma_start(out=x_sbs[0][:, k, 512:S], in_=xr[0, bass_ts(k, 128), 512:S])
                nc.sync.dma_start(out=wv_sb[:, k, :], in_=wv_r[:, k, :])
            for k in range(KT):
                nc.sync.dma_start(out=wqk_sb[:, k, 256 : 2 * F], in_=wqk_r[:, k, 256 : 2 * F])
            for b in range(1, BL):
                for k in range(KT):
                    nc.sync.dma_start(out=x_sbs[b][:, k, :], in_=xr[b, bass_ts(k, 128), :])
            nc.sync.dma_start(out=wo_sb, in_=wo_r)

            b_qk = bias_sb[:, 0 : 2 * NH]
            b_v = bias_sb[:, 2 * NH : 2 * NH + F]
            b_out = bias_sb[:, 2 * NH + F : 2 * NH + F + KT]
            b_shift = bias_sb[:, 2 * NH + F + KT :]

            rep_ctx = tc.For_i(0, repeat, 1) if repeat > 1 else contextlib.nullcontext()
            with rep_ctx:
                _body(
                    nc, tc, x_sbs, qkp, qrp, vp, rp, epool, v8pool, smallp, otp,
                    pp, ps, pr, wqk_sb, wv_sb, wo_sb, b_qk, b_v, b_out, b_shift, out_d,
                    F32, F32R, BF16, FP8, EXP, DR, ADD, ts,
                )

    nc.compile()
    return nc


def _body(
    nc, tc, x_sbs, qkp, qrp, vp, rp, epool, v8pool, smallp, otp,
    pp, ps, pr, wqk_sb, wv_sb, wo_sb, b_qk, b_v, b_out, b_shift, out_d,
    F32, F32R, BF16, FP8, EXP, DR, ADD, ts,
):
    qk_sb = [qkp.tile([128, 2 * NH, S], FP8, name=f"qksb{b}", tag="qksb") for b in range(BL)]
    qk8 = [qrp.tile([64, 2 * NH, 2, S], FP8, name=f"qk8_{b}", tag="qk8") for b in range(BL)]
    v_sb = [vp.tile([128, ST, F], BF16, name=f"vsb{b}", tag="vsb") for b in range(BL)]
    resT = [rp.tile([128, NH, S], F32R, name=f"resT{b}", tag="resT") for b in range(BL)]

    def qk_chunk(b, t, n):
        def emit():
            acc = pp.tile([128, 512], F32, name="pacc", tag="pp")
            for k in range(KT):
                nc.tensor.matmul(
                    acc, wqk_sb[:, k, ts(t, 128)], x_sbs[b][:, k, ts(n, 512)],
                    start=(k == 0), stop=(k == KT - 1),
                )
            nc.vector.tensor_scalar_add(qk_sb[b][:, t, ts(n, 512)], acc, b_qk[:, t : t + 1])
            if n == NT - 1:
                # SB->SB reshuffle: partitions (0:64, 64:128) -> planes of
                # qk8[b][:, t]. Split into 4 transfers to cut per-queue latency.
                for half in range(2):
                    src = qk_sb[b][64 * half : 64 * half + 64, t, :]
                    for piece in range(2):
                        nc.sync.dma_start(
                            out=qk8[b][:, t, half, ts(piece, 512)],
                            in_=src[:, ts(piece, 512)],
                        )
        return emit

    def v_chunk(b, st):
        def emit():
            acc = pp.tile([128, 512], F32, name="pacc", tag="pp")
            for k in range(KT):
                nc.tensor.matmul(
                    acc, x_sbs[b][:, k, ts(st, 128)], wv_sb[:, k, :],
                    start=(k == 0), stop=(k == KT - 1),
                )
            nc.vector.tensor_add(v_sb[b][:, st, :], acc, b_v)
        return emit

    def out_chunk(b, ct, n, eng=None):
        def emit():
            acc = pp.tile([128, 512], F32, name="pacc", tag="pp")
            for hh in range(NH):
                nc.tensor.matmul(
                    acc, wo_sb[:, hh, ts(ct, 128)],
                    resT[b][:, hh, ts(n, 512)],
                    start=(hh == 0), stop=(hh == NH - 1),
                )
            ot = otp.tile([128, 512], F32, name="ot", tag="ot")
            nc.vector.scalar_tensor_tensor(
                ot, acc, b_out[:, ct : ct + 1],
                x_sbs[b][:, ct, ts(n, 512)].bitcast(F32), ADD, ADD,
            )
            # 4-way split keeps per-queue DMA latency ~3us so ot bufs recycle
            for piece in range(4):
                nc.sync.dma_start(
                    out=out_d[b, ts(ct, 128), n * 512 + piece * 128 : n * 512 + piece * 128 + 128],
                    in_=ot[:, ts(piece, 128)],
                )
        return emit

    # ---- static filler schedule (PE work slotted into exp-steps) ----
    # Deadlines: qk8(b,h-tiles) must land ~1.5 steps before that head's first
    # scores; v tiles before their v_sc; out-proj(b0) after resT(b0) done.
    fillers: dict = {}

    def put(s, *ems):
        fillers.setdefault(s, []).extend(ems)

    put(0, v_chunk(0, 2), v_chunk(0, 3))
    put(1, qk_chunk(0, 2, 0))
    put(2, qk_chunk(0, 2, 1))
    put(3, qk_chunk(0, 3, 0))
    put(4, qk_chunk(0, 3, 1))
    put(5, v_chunk(0, 4))
    put(6, v_chunk(0, 5))
    put(7, v_chunk(0, 6))
    put(8, v_chunk(0, 7))
    put(10, qk_chunk(0, 4, 0))
    put(11, qk_chunk(0, 4, 1))
    put(12, qk_chunk(0, 5, 0))
    put(13, qk_chunk(0, 5, 1))
    put(16, qk_chunk(0, 6, 0))
    put(17, qk_chunk(0, 6, 1))
    put(18, qk_chunk(0, 7, 0))
    put(19, qk_chunk(0, 7, 1))
    put(20, qk_chunk(1, 0, 0))
    put(21, qk_chunk(1, 0, 1))
    put(22, qk_chunk(1, 1, 0))
    put(23, qk_chunk(1, 1, 1))
    for st in range(ST):
        put(24 + st, v_chunk(1, st))
    put(32, qk_chunk(1, 2, 0))
    put(33, qk_chunk(1, 2, 1))
    put(34, qk_chunk(1, 3, 0))
    put(35, qk_chunk(1, 3, 1))
    put(40, qk_chunk(1, 4, 0))
    put(41, qk_chunk(1, 4, 1))
    put(42, qk_chunk(1, 5, 0))
    put(43, qk_chunk(1, 5, 1))
    put(44, qk_chunk(1, 6, 0))
    put(45, qk_chunk(1, 6, 1))
    put(46, qk_chunk(1, 7, 0))
    put(47, qk_chunk(1, 7, 1))
    for i, (ct, n) in enumerate([(c, n) for c in range(KT) for n in range(NT)]):
        put(48 + i, out_chunk(0, ct, n))

    # ---- the 64-step pipeline ----
    steps = [(b, h, jt) for b in range(BL) for h in range(NH) for jt in range(ST)]
    pairs: dict = {}
    raccs: dict = {}
    ssums: dict = {}

    def emit_av(b, h, jtp):
        e8p, v8p = pairs.pop((b, h, jtp))
        if jtp == 0:
            raccs[(b, h)] = pr.tile([128, S], F32, name="racc", tag="racc")
        racc = raccs[(b, h)]
        for n in range(NT):
            nc.tensor.matmul(
                racc[:, ts(n, 512)], v8p, e8p[:, :, ts(n, 512)],
                start=(jtp == 0), stop=(jtp == ST // 2 - 1), perf_mode=DR,
            )
        if jtp == ST // 2 - 1:
            for n in range(NT):
                nc.vector.tensor_copy(resT[b][:, h, ts(n, 512)], racc[:, ts(n, 512)])

    def emit_vsc(s):
        # recip + v scaling for step s, emitted 2 steps late so the v_proj
        # fillers for that tile have already been emitted (program order
        # defines the dataflow).
        if s < 0 or s >= len(steps):
            return
        tb, th, tjt = steps[s]
        e8p, v8p = pairs[(tb, th, tjt // 2)]
        ssum = ssums.pop(s)
        nc.vector.reciprocal(ssum[:, 1:2], ssum[:, 0:1])
        nc.gpsimd.tensor_scalar_mul(
            v8p[:, tjt % 2, :], v_sb[tb][:, tjt, ts(th, DK)], ssum[:, 1:2]
        )

    def flush_av(s):
        tgt = s - 2  # AV pair lags its last exp by 2 steps
        if tgt < 0 or tgt >= len(steps):
            return
        tb, th, tjt = steps[tgt]
        if tjt % 2 == 1:
            emit_av(tb, th, tjt // 2)

    # prologue: first head's q/k projection + first two v tiles
    qk_chunk(0, 1, 0)()
    qk_chunk(0, 1, 1)()
    qk_chunk(0, 0, 0)()
    qk_chunk(0, 0, 1)()
    v_chunk(0, 0)()
    v_chunk(0, 1)()

    for s, (b, h, jt) in enumerate(steps):
        sacc = ps.tile([128, S], F32, name="sacc", tag="sacc")
        for n in range(NT):
            nc.tensor.matmul(
                sacc[:, ts(n, 512)],
                qk8[b][:, 2 * h + 1, :, ts(jt, 128)],
                qk8[b][:, 2 * h, :, ts(n, 512)],
                start=True, stop=True, perf_mode=DR,
            )
        jtp, parity = jt // 2, jt % 2
        if parity == 0:
            pairs[(b, h, jtp)] = (
                epool.tile([128, 2, S], FP8, name="e8", tag="e8"),
                v8pool.tile([128, 2, DK], FP8, name="v8", tag="v8"),
            )
        e8p, v8p = pairs[(b, h, jtp)]
        ssum = smallp.tile([128, 2], F32, name="ssum", tag="ssum")
        ssums[s] = ssum
        nc.scalar.activation(
            out=e8p[:, parity, :], in_=sacc, func=EXP,
            scale=SCALE, bias=b_shift, accum_out=ssum[:, 0:1],
        )
        emit_vsc(s - 2)
        flush_av(s)
        for f in fillers.get(s, []):
            f()

    for s in (len(steps), len(steps) + 1):
        emit_vsc(s - 2)
        flush_av(s)

    # tail: last batch's output projection
    for ct in range(KT):
        for n in range(NT):
            out_chunk(1, ct, n)()


def bass_ts(i, size):
    import concourse.bass as bass

    return bass.ts(i, size)


def _prep_inputs(x, w_proj, b_proj, w_out, b_out):
    """Host-side reshaping into the layouts the kernel expects."""
    x_f = np.ascontiguousarray(x.reshape(B, C, S), dtype=np.float32)
    wT = np.asarray(w_proj, dtype=np.float32).T  # [C, 3*F], f = h*384 + j
    w_qkT = np.concatenate(
        [wT[:, h * 384 : h * 384 + 256] for h in range(NH)], axis=1
    )  # [C, 2F]; col tile t=2h -> q_h, t=2h+1 -> k_h
    w_vT = VSC * np.concatenate(
        [wT[:, h * 384 + 256 : h * 384 + 384] for h in range(NH)], axis=1
    )  # [C, F], pre-scaled so v*inv_den stays in fp8 range
    w_outT = np.ascontiguousarray(np.asarray(w_out, dtype=np.float32).T / VSC)  # [F, C]
    b_proj = np.asarray(b_proj, dtype=np.float32)
    b_qk = np.stack(
        [
            b_proj[h * 384 + half * 128 : h * 384 + half * 128 + 128]
            for h in range(NH)
            for half in range(2)
        ],
        axis=1,
    )  # [128, 2*NH], col t matches qk tile order
    b_v = VSC * np.concatenate(
        [b_proj[h * 384 + 256 : h * 384 + 384] for h in range(NH)]
    )  # [F]
    b_v_bcast = np.broadcast_to(b_v, (128, F))
    b_out_t = np.asarray(b_out, dtype=np.float32).reshape(KT, 128).T  # [128, KT]
    shift_col = np.full((128, 1), ESHIFT, dtype=np.float32)
    bias = np.ascontiguousarray(
        np.concatenate([b_qk, b_v_bcast, b_out_t, shift_col], axis=1), dtype=np.float32
    )  # [128, 2*NH + F + KT + 1]
    return x_f, np.ascontiguousarray(w_qkT), np.ascontiguousarray(w_vT), w_outT, bias


def kernel(x, w_proj, b_proj, w_out, b_out, n_heads):
    from concourse.bass_utils import run_bass_kernel_spmd

    assert int(n_heads) == NH
    x_f, w_qkT, w_vT, w_outT, bias = _prep_inputs(x, w_proj, b_proj, w_out, b_out)

    if "nc" not in _CACHE:
        _CACHE["nc"] = _build()
    nc = _CACHE["nc"]

    in_maps = [
        {
            "x": np.ascontiguousarray(x_f[c * BL : (c + 1) * BL]),
            "w_qkT": w_qkT,
            "w_vT": w_vT,
            "w_outT": w_outT,
            "bias": bias,
        }
        for c in range(NCORES)
    ]
    res = run_bass_kernel_spmd(nc, in_maps, list(range(NCORES)))
    out = np.concatenate([res.results[c]["out"] for c in range(NCORES)], axis=0)
    return out.reshape(B, C, 32, 32)


# revision 9
# speedup vs baseline: 1.6510x; 1.6510x over previous
"""AttentionBlock Trainium2 kernel (v3).

Reference computation (B=16, C=512, H=W=32, n_heads=4, d_k=128):
    xs   = x.reshape(B,C,S).T            # [B, S, C],  S = 1024
    qkv  = xs @ w_proj.T + b_proj        # [B, S, 1536]
    S_   = einsum('bihd,bjhd->bijh', q, k) * d_k**-0.5
    attn = softmax(S_, axis=1)           # over the QUERY axis i (source quirk)
    res  = einsum('bijh,bjhd->bihd', attn, v)
    out  = res @ w_out.T + b_out + xs    # residual
    return out.T.reshape(B, C, H, W)

Data-parallel over batch, 2 batches per core on 8 cores.

HW cost model (measured via microbenchmarks on this setup; ldw-opt is
disabled so every matmul pays a serial LdWeights):
    matmul ~= 50ns + 0.417ns * (ldw_cols * dtype_bytes + out_cols)
    fp8 [128,128]x[128,512]:               270 ns
    fp8 DoubleRow [128,2,128]x[128,2,512]: 370 ns (256-wide contraction)
    f32r [128,128]x[128,512]:             ~476 ns (4-byte ldw!)
    ACT Exp [128,1024] + accum:           1362 ns
    DVE psum->fp8 [128,512]:              ~898 ns; psum->f32/bf16 ~700 ns
So ALL matmuls run fp8: scores non-DR (contraction is d_k=128), the
projections and AV in DoubleRow (256-wide contraction halves instruction
count). Back-to-back accumulation into the same psum bank stalls (~+300ns),
so chunk pairs alternate psum banks A/B.

Engine budget per core (2 batches): PE ~105us (the wall), ACT 87us,
DVE ~73us, Pool ~22us. Emitted as a 64-step software pipeline (one exp per
step) with projection/output units statically placed to meet dataflow
deadlines.

Numerics (numpy sim of this quantization chain: rel err ~1.3e-2 vs the 2e-2
gate; HW has measured below sim): exp(s*scale - 3) keeps e in fp8 range;
v8 = (v * inv_den) * 1024; resT8 = racc/64; wo8 = 512*w_out.T;
out = psum/8192 + (x + b_out)  [b_out folded into the f32 residual input
on the host].
"""
import sys

for _p in (
    "/opt/trn_rl_repo",
    "/root/.axon_site",
    "/root/.axon_site/_ro/trn_rl_repo",
    "/root/.axon_site/_ro/pypackages",
):
    if _p not in sys.path:
        sys.path.append(_p)

import numpy as np

B = 16
C = 512
S = 1024  # H*W
NH = 4
DK = 128
F = NH * DK  # 512
NCORES = 8
BL = B // NCORES  # batches per core
KT = C // 128  # 4  contraction tiles over channels
ST = S // 128  # 8  seq tiles
NT = S // 512  # 2  free-dim chunks of 512
SCALE = float(DK) ** -0.5
ESHIFT = -3.0   # exp(s*SCALE + ESHIFT): keeps fp8 e <= ~45
VSC = 1024.0    # v8 = (v * inv_den) * VSC
RSC = 1.0 / 64.0   # resT8 = racc * RSC -> 16*res
WOSC = 512.0    # wo8 = fp8(WOSC * w_out.T)
OSC = 1.0 / (VSC * RSC * WOSC)  # out = psum * OSC + (x + b_out)

_CACHE: dict = {}


def _build(repeat=1):
    """Build the kernel. repeat>1 wraps the per-call workload in an on-device
    For_i loop (timing only: amortizes the ~10ms axon dispatch)."""
    import contextlib

    import concourse.bass as bass
    import concourse.tile as tile
    from concourse import bacc, mybir

    F32 = mybir.dt.float32
    BF16 = mybir.dt.bfloat16
    FP8 = mybir.dt.float8e4
    U8 = mybir.dt.uint8
    EXP = mybir.ActivationFunctionType.Exp
    DR = mybir.MatmulPerfMode.DoubleRow
    ADD = mybir.AluOpType.add
    MULT = mybir.AluOpType.mult
    ts = bass.ts

    nc = bacc.Bacc("TRN2", debug=False)
    # xb = x + b_out (host-folded); x8/w*8 are e4m3 bytes shipped as uint8
    xb_d = nc.dram_tensor("xb", [BL, C, S], F32, kind="ExternalInput").ap()
    x8_d = nc.dram_tensor("x8", [BL, C, S], U8, kind="ExternalInput").ap()
    wqk_d = nc.dram_tensor("wqk8", [C, 2 * F], U8, kind="ExternalInput").ap()
    wv_d = nc.dram_tensor("wv8", [C, F], U8, kind="ExternalInput").ap()
    wo_d = nc.dram_tensor("wo8", [F, C], U8, kind="ExternalInput").ap()
    bias_d = nc.dram_tensor("bias", [128, 2 * NH + F + 1], F32, kind="ExternalInput").ap()
    out_d = nc.dram_tensor("out", [BL, C, S], F32, kind="ExternalOutput").ap()

    x8r = x8_d.bitcast(FP8)
    wqk_r = wqk_d.rearrange("(k p) m -> p k m", p=128).bitcast(FP8)
    wv_r = wv_d.rearrange("(k p) m -> p k m", p=128).bitcast(FP8)
    wo_r = wo_d.rearrange("(k p) m -> p k m", p=128).bitcast(FP8)

    with tile.TileContext(nc) as tc:
        with (
            tc.tile_pool(name="const", bufs=1) as constp,
            tc.tile_pool(name="xp", bufs=2) as xp,
            tc.tile_pool(name="qkp", bufs=2) as qkp,
            tc.tile_pool(name="vp", bufs=2) as vp,
            tc.tile_pool(name="rp", bufs=2) as rp,
            tc.tile_pool(name="ep", bufs=3) as epool,
            tc.tile_pool(name="v8p", bufs=4) as v8pool,
            tc.tile_pool(name="small", bufs=8) as smallp,
            tc.tile_pool(name="otp", bufs=4) as otp,
            # psum: pp = [128,512]x2 proj banks (A/B alternation);
            # ps = [128,1024]x2 score tiles; pr = [128,1024]x1 AV accumulator.
            tc.tile_pool(name="pp", bufs=2, space="PSUM") as pp,
            tc.tile_pool(name="ps", bufs=2, space="PSUM") as ps,
            tc.tile_pool(name="pr", bufs=1, space="PSUM") as pr,
        ):
            wqk_sb = constp.tile([128, KT, 2 * F], FP8)
            wv_sb = constp.tile([128, KT, F], FP8)
            wo_sb = constp.tile([128, KT, C], FP8)
            bias_sb = constp.tile([128, 2 * NH + F + 1], F32)
            xb_sbs = [xp.tile([128, KT, S], F32, name=f"xb{b}", tag="xb") for b in range(BL)]
            x8_sbs = [xp.tile([128, KT, S], FP8, name=f"x8_{b}", tag="x8") for b in range(BL)]

            # DMA order: bias first, then b0's fp8 x + qk weights (gate the
            # prologue), wv + b1 fp8 x, wo, the f32 residual xb last.
            nc.sync.dma_start(out=bias_sb, in_=bias_d)
            for k in range(KT):
                nc.sync.dma_start(out=x8_sbs[0][:, k, :], in_=x8r[0, bass_ts(k, 128), :])
                nc.sync.dma_start(out=wqk_sb[:, k, :], in_=wqk_r[:, k, :])
            for k in range(KT):
                nc.sync.dma_start(out=wv_sb[:, k, :], in_=wv_r[:, k, :])
                nc.sync.dma_start(out=x8_sbs[1][:, k, :], in_=x8r[1, bass_ts(k, 128), :])
            nc.sync.dma_start(out=wo_sb, in_=wo_r)
            for b in range(BL):
                for k in range(KT):
                    nc.sync.dma_start(out=xb_sbs[b][:, k, :], in_=xb_d[b, bass_ts(k, 128), :])

            b_qk = bias_sb[:, 0 : 2 * NH]
            b_v = bias_sb[:, 2 * NH : 2 * NH + F]
            b_shift = bias_sb[:, 2 * NH + F :]

            rep_ctx = tc.For_i(0, repeat, 1) if repeat > 1 else contextlib.nullcontext()
            with rep_ctx:
                _body(
                    nc, xb_sbs, x8_sbs, qkp, vp, rp, epool, v8pool, smallp, otp,
                    pp, ps, pr, wqk_sb, wv_sb, wo_sb, b_qk, b_v, b_shift, out_d,
                    F32, BF16, FP8, EXP, DR, ADD, MULT, ts,
                )

    nc.compile()
    return nc


def _body(
    nc, xb_sbs, x8_sbs, qkp, vp, rp, epool, v8pool, smallp, otp,
    pp, ps, pr, wqk_sb, wv_sb, wo_sb, b_qk, b_v, b_shift, out_d,
    F32, BF16, FP8, EXP, DR, ADD, MULT, ts,
):
    qk_sb = [qkp.tile([128, 2 * NH, S], FP8, name=f"qksb{b}", tag="qksb") for b in range(BL)]
    v_sb = [vp.tile([128, ST, F], BF16, name=f"vsb{b}", tag="vsb") for b in range(BL)]
    resT8 = [rp.tile([128, NH, S], FP8, name=f"resT{b}", tag="resT") for b in range(BL)]

    # ---- filler units: 4 DoubleRow matmuls each, A/B bank alternation ----
    def qk_unit(b, t):
        # q/k f-tile t: qk_sb[b][:, t, :] = fp8(w_qkT[:, t].T @ x + b_qk[t])
        def emit():
            accA = pp.tile([128, 512], F32, name="pA", tag="pp")
            accB = pp.tile([128, 512], F32, name="pB", tag="pp")
            for kp in range(KT // 2):
                w_pair = wqk_sb[:, 2 * kp : 2 * kp + 2, ts(t, 128)]
                for n, acc in ((0, accA), (1, accB)):
                    nc.tensor.matmul(
                        acc, w_pair, x8_sbs[b][:, 2 * kp : 2 * kp + 2, ts(n, 512)],
                        start=(kp == 0), stop=(kp == KT // 2 - 1), perf_mode=DR,
                    )
            for n, acc in ((0, accA), (1, accB)):
                nc.vector.tensor_scalar_add(
                    qk_sb[b][:, t, ts(n, 512)], acc, b_qk[:, t : t + 1]
                )
        return emit

    def v_unit(b, stp):
        # V rows for s-tiles (2*stp, 2*stp+1): v_sb = x.T @ w_vT + b_v (bf16)
        def emit():
            accA = pp.tile([128, 512], F32, name="pA", tag="pp")
            accB = pp.tile([128, 512], F32, name="pB", tag="pp")
            for kp in range(KT // 2):
                kpair = slice(2 * kp, 2 * kp + 2)
                for st, acc in ((2 * stp, accA), (2 * stp + 1, accB)):
                    nc.tensor.matmul(
                        acc, x8_sbs[b][:, kpair, ts(st, 128)], wv_sb[:, kpair, :],
                        start=(kp == 0), stop=(kp == KT // 2 - 1), perf_mode=DR,
                    )
            for st, acc in ((2 * stp, accA), (2 * stp + 1, accB)):
                nc.vector.tensor_add(v_sb[b][:, st, :], acc, b_v)
        return emit

    def out_unit(b, ct):
        # out c-tile ct: psum = wo8[:, ct].T @ resT8 ; out = psum*OSC + xb
        def emit():
            accA = pp.tile([128, 512], F32, name="pA", tag="pp")
            accB = pp.tile([128, 512], F32, name="pB", tag="pp")
            for hp in range(NH // 2):
                wo_pair = wo_sb[:, 2 * hp : 2 * hp + 2, ts(ct, 128)]
                for n, acc in ((0, accA), (1, accB)):
                    nc.tensor.matmul(
                        acc, wo_pair, resT8[b][:, 2 * hp : 2 * hp + 2, ts(n, 512)],
                        start=(hp == 0), stop=(hp == NH // 2 - 1), perf_mode=DR,
                    )
            for n, acc in ((0, accA), (1, accB)):
                ot = otp.tile([128, 512], F32, name="ot", tag="ot")
                nc.vector.scalar_tensor_tensor(
                    ot, acc, OSC, xb_sbs[b][:, ct, ts(n, 512)], MULT, ADD,
                )
                for piece in range(4):
                    nc.sync.dma_start(
                        out=out_d[b, ts(ct, 128), n * 512 + piece * 128 : n * 512 + piece * 128 + 128],
                        in_=ot[:, ts(piece, 128)],
                    )
        return emit

    # ---- static filler schedule (dataflow deadlines in comments) ----
    fillers: dict = {}

    def put(s, *ems):
        fillers.setdefault(s, []).extend(ems)

    put(0, v_unit(0, 1))     # v st2,3 needed by v_sc at s4-5
    put(1, qk_unit(0, 2))    # h1 scores start s8
    put(2, v_unit(0, 2))
    put(3, qk_unit(0, 3))
    put(4, v_unit(0, 3))
    put(6, qk_unit(0, 4))    # h2 scores start s16
    put(8, qk_unit(0, 5))
    put(10, qk_unit(0, 6))   # h3 scores start s24
    put(12, qk_unit(0, 7))
    put(14, qk_unit(1, 0))   # b1 h0 scores start s32
    put(16, qk_unit(1, 1))
    put(18, v_unit(1, 0))
    put(20, v_unit(1, 1))
    put(22, v_unit(1, 2))
    put(24, v_unit(1, 3))
    put(26, qk_unit(1, 2))   # b1 h1 scores start s40
    put(28, qk_unit(1, 3))
    put(30, qk_unit(1, 4))   # b1 h2 start s48
    put(33, qk_unit(1, 5))
    put(35, qk_unit(1, 6))   # b1 h3 start s56
    put(37, qk_unit(1, 7))
    put(40, out_unit(0, 0))  # resT8(b0) complete ~s35
    put(43, out_unit(0, 1))
    put(46, out_unit(0, 2))
    put(49, out_unit(0, 3))

    # ---- the 64-step pipeline ----
    steps = [(b, h, jt) for b in range(BL) for h in range(NH) for jt in range(ST)]
    pairs: dict = {}
    raccs: dict = {}
    ssums: dict = {}

    def emit_av(b, h, jtp):
        e8p, v8p = pairs.pop((b, h, jtp))
        if jtp == 0:
            raccs[(b, h)] = pr.tile([128, S], F32, name="racc", tag="racc")
        racc = raccs[(b, h)]
        for n in range(NT):
            nc.tensor.matmul(
                racc[:, ts(n, 512)], v8p, e8p[:, :, ts(n, 512)],
                start=(jtp == 0), stop=(jtp == ST // 2 - 1), perf_mode=DR,
            )
        if jtp == ST // 2 - 1:
            for n in range(NT):
                nc.vector.tensor_scalar_mul(
                    resT8[b][:, h, ts(n, 512)], racc[:, ts(n, 512)], RSC
                )

    def emit_vsc(s):
        # recip + v scaling for step s, emitted 2 steps late so the v_proj
        # units for that tile are already emitted (program order = dataflow).
        if s < 0 or s >= len(steps):
            return
        tb, th, tjt = steps[s]
        e8p, v8p = pairs[(tb, th, tjt // 2)]
        ssum = ssums.pop(s)
        nc.vector.reciprocal(ssum[:, 1:2], ssum[:, 0:1])
        nc.gpsimd.tensor_scalar(
            v8p[:, tjt % 2, :], v_sb[tb][:, tjt, ts(th, DK)],
            ssum[:, 1:2], VSC, MULT, MULT,
        )

    def flush_av(s):
        tgt = s - 2  # AV pair lags its last exp by 2 steps
        if tgt < 0 or tgt >= len(steps):
            return
        tb, th, tjt = steps[tgt]
        if tjt % 2 == 1:
            emit_av(tb, th, tjt // 2)

    # prologue: first head's q/k tiles + first two v tiles
    qk_unit(0, 1)()
    qk_unit(0, 0)()
    v_unit(0, 0)()

    for s, (b, h, jt) in enumerate(steps):
        sacc = ps.tile([128, S], F32, name="sacc", tag="sacc")
        for n in range(NT):
            nc.tensor.matmul(
                sacc[:, ts(n, 512)],
                qk_sb[b][:, 2 * h + 1, ts(jt, 128)],
                qk_sb[b][:, 2 * h, ts(n, 512)],
                start=True, stop=True,
            )
        jtp, parity = jt // 2, jt % 2
        if parity == 0:
            pairs[(b, h, jtp)] = (
                epool.tile([128, 2, S], FP8, name="e8", tag="e8"),
                v8pool.tile([128, 2, DK], FP8, name="v8", tag="v8"),
            )
        e8p, v8p = pairs[(b, h, jtp)]
        ssum = smallp.tile([128, 2], F32, name="ssum", tag="ssum")
        ssums[s] = ssum
        nc.scalar.activation(
            out=e8p[:, parity, :], in_=sacc, func=EXP,
            scale=SCALE, bias=b_shift, accum_out=ssum[:, 0:1],
        )
        emit_vsc(s - 2)
        flush_av(s)
        for f in fillers.get(s, []):
            f()

    for s in (len(steps), len(steps) + 1):
        emit_vsc(s - 2)
        flush_av(s)

    # tail: last batch's output projection
    for ct in range(KT):
        out_unit(1, ct)()


def bass_ts(i, size):
    import concourse.bass as bass

    return bass.ts(i, size)


def _fp8_bytes(a):
    import ml_dtypes

    return np.ascontiguousarray(
        np.asarray(a, dtype=np.float32).astype(ml_dtypes.float8_e4m3).view(np.uint8)
    )


def _prep_inputs(x, w_proj, b_proj, w_out, b_out):
    """Host-side quantization + reshaping into the layouts the kernel expects."""
    x_f = np.ascontiguousarray(x.reshape(B, C, S), dtype=np.float32)
    xb = x_f + np.asarray(b_out, dtype=np.float32)[None, :, None]  # residual + b_out
    x8 = _fp8_bytes(x_f)
    wT = np.asarray(w_proj, dtype=np.float32).T  # [C, 3*F], f = h*384 + j
    w_qkT = np.concatenate(
        [wT[:, h * 384 : h * 384 + 256] for h in range(NH)], axis=1
    )  # [C, 2F]; col tile t=2h -> q_h, t=2h+1 -> k_h
    w_vT = np.concatenate(
        [wT[:, h * 384 + 256 : h * 384 + 384] for h in range(NH)], axis=1
    )  # [C, F]
    w_outT = WOSC * np.asarray(w_out, dtype=np.float32).T  # [F, C]
    b_proj = np.asarray(b_proj, dtype=np.float32)
    b_qk = np.stack(
        [
            b_proj[h * 384 + half * 128 : h * 384 + half * 128 + 128]
            for h in range(NH)
            for half in range(2)
        ],
        axis=1,
    )  # [128, 2*NH], col t matches qk tile order
    b_v = np.concatenate(
        [b_proj[h * 384 + 256 : h * 384 + 384] for h in range(NH)]
    )  # [F]
    b_v_bcast = np.broadcast_to(b_v, (128, F))
    shift_col = np.full((128, 1), ESHIFT, dtype=np.float32)
    bias = np.ascontiguousarray(
        np.concatenate([b_qk, b_v_bcast, shift_col], axis=1), dtype=np.float32
    )  # [128, 2*NH + F + 1]
    return xb, x8, _fp8_bytes(w_qkT), _fp8_bytes(w_vT), _fp8_bytes(w_outT), bias


def kernel(x, w_proj, b_proj, w_out, b_out, n_heads):
    from concourse.bass_utils import run_bass_kernel_spmd

    assert int(n_heads) == NH
    xb, x8, wqk8, wv8, wo8, bias = _prep_inputs(x, w_proj, b_proj, w_out, b_out)

    if "nc" not in _CACHE:
        _CACHE["nc"] = _build()
    nc = _CACHE["nc"]

    in_maps = [
        {
            "xb": np.ascontiguousarray(xb[c * BL : (c + 1) * BL]),
            "x8": np.ascontiguousarray(x8[c * BL : (c + 1) * BL]),
            "wqk8": wqk8,
            "wv8": wv8,
            "wo8": wo8,
            "bias": bias,
        }
        for c in range(NCORES)
    ]
    res = run_bass_kernel_spmd(nc, in_maps, list(range(NCORES)))
    out = np.concatenate([res.results[c]["out"] for c in range(NCORES)], axis=0)
    return out.reshape(B, C, 32, 32)
